# revision 48
# baseline (speedup 1.0000x reference)
"""Causal self-attention Trainium2 Bass kernel (8-core SPMD).

Problem: x[4,2048,1024] -> CausalSelfAttention(n_head=16) -> [4,2048,1024], f32.

Sharding: 8 cores = 4 batches x 2 head-groups (8 heads each). No collectives:
each core computes its head-group's partial output projection; the host sums
the two partials per batch and adds the folded bias.

Per-core dataflow (all matmuls fp16-in/f32-psum, transpose-free):
  All inputs (xT, w_qk, w_v, w_pr) are loaded ONCE into SBUF with large
  descriptors; steady state does no input DMA.
  boot:    QKV for token chunk 0 with kc-outer accumulation across 8 PSUM
           banks so the PE starts as soon as the first (w, x) slices land.
  phase 1: qT = (w_q*s).T @ xT + b_q*s   [512,2048]  feature-major
           kT = w_k.T @ xT               [512,2048]  (b_k dropped: softmax
                                                      row-shift invariance)
           V  = xT.T @ w_v               [2048,512]  token-major, augmented
                                                     with a ones column
  phase 2: per (q-chunk j, head pair): S^T = kT.T-slice @ qT-slice (k on
           partitions), additive causal mask on diagonal 128x128 squares,
           P = exp(S^T) (no max subtraction; scores are O(3)), PV^T
           accumulated over k-tiles with the ones column producing row sums r
           in row 64; normalize via a rank-2 PE broadcast of r (no DRAM
           bounce), reciprocal, and PSUM-side multiplies; then
           Y^T += w_pr.T @ OUT^T, with proj(j) interleaved into
           attention(j+1) as PE filler. The causal mask is applied
           multiplicatively on P (0/1 triangle) after exp.
Host folds b_v and b_proj into one constant vector: b_v @ w_proj + b_proj
(softmax rows sum to 1, so the V bias passes through exactly).
"""

import sys

if "/opt/trn_rl_repo" not in sys.path:
    sys.path.insert(0, "/opt/trn_rl_repo")

from contextlib import ExitStack

import numpy as np

import concourse.bass as bass
import concourse.tile as tile
from concourse import bacc, mybir
from concourse.bass_utils import run_bass_kernel_spmd

F32 = mybir.dt.float32
F16 = mybir.dt.float16
AFT = mybir.ActivationFunctionType

C = 1024          # n_embd
T = 2048          # seq len
NB = 4            # batch
NHEAD = 16
HD = 64           # head dim
HPG = 8           # heads per group (per core)
GC = HPG * HD     # 512 features per group
NKC = C // 128    # 8 contraction tiles over C
NJ = T // 512     # 4 q-chunks of 512


def bcast(ap, n, axis):
    """Insert a step-0 (broadcast) dim of size n at free-dim position axis."""
    steps = list(ap.ap)
    steps.insert(axis, [0, n])
    return bass.AP(tensor=ap.tensor, offset=ap.offset, ap=steps)


def build_kernel(nc: bass.Bass):
    xT = nc.dram_tensor("xT", [C, T], F16, kind="ExternalInput")
    w_qk = nc.dram_tensor("w_qk", [C, 2 * GC], F16, kind="ExternalInput")
    w_v = nc.dram_tensor("w_v", [C, GC], F16, kind="ExternalInput")
    b_q = nc.dram_tensor("b_q", [GC], F32, kind="ExternalInput")
    w_pr = nc.dram_tensor("w_pr", [GC, C], F16, kind="ExternalInput")
    tri16 = nc.dram_tensor("tri16", [128, 128], F16, kind="ExternalInput")
    sel2 = nc.dram_tensor("sel2", [64, 128], F16, kind="ExternalInput")
    yT = nc.dram_tensor("yT", [C, T], F16, kind="ExternalOutput")

    xT_r = xT.rearrange("(kc p) t -> p kc t", p=128)
    w_qk_r = w_qk.rearrange("(kc p) c -> p kc c", p=128)
    w_pr_r = w_pr.rearrange("(kc p) c -> p kc c", p=128)

    with tile.TileContext(nc) as tc, ExitStack() as ctx:
        persist = ctx.enter_context(tc.tile_pool(name="persist", bufs=1))
        qpool = ctx.enter_context(tc.tile_pool(name="qpool", bufs=2))
        ppool = ctx.enter_context(tc.tile_pool(name="ppool", bufs=4))
        rpool = ctx.enter_context(tc.tile_pool(name="rpool", bufs=2))
        prhs_pool = ctx.enter_context(tc.tile_pool(name="prhsp", bufs=2))
        ypool = ctx.enter_context(tc.tile_pool(name="ypool", bufs=3))
        pshare = ctx.enter_context(
            tc.tile_pool(name="pshare", bufs=2, space="PSUM")
        )
        psum_s = ctx.enter_context(
            tc.tile_pool(name="psum_s", bufs=2, space="PSUM")
        )
        psum_o = ctx.enter_context(
            tc.tile_pool(name="psum_o", bufs=1, space="PSUM")
        )

        # per-chunk K^T and V tiles (separate tiles so chunk n+1 writes
        # never alias chunk <=n reads during cross-phase interleaving)
        ktc = [persist.tile([128, 4, 512], F16, tag=f"kt{n}", name=f"kt{n}")
               for n in range(NJ)]
        vc = [persist.tile([128, 4, HPG, HD + 1], F16, tag=f"v{n}",
                           name=f"v{n}") for n in range(NJ)]
        xT_s = persist.tile([128, NKC, T], F16, tag="xs")
        wqk_s = persist.tile([128, NKC, 2 * GC], F16, tag="wqk")
        wv_s = persist.tile([128, NKC, GC], F16, tag="wv")
        wpr_s = persist.tile([128, 4, C], F16, tag="wpr")
        b_q_s = persist.tile([128, 4], F32, tag="bq")
        tri_s = persist.tile([128, 128], F16, tag="tri")
        ones_s = persist.tile([128, 4 * HPG], F32, tag="ones")
        sel_s = persist.tile([64, 128], F16, tag="sel")
        rr_s = persist.tile([64, 512], F16, tag="rr")

        def emit_input_dmas():
            # tiny consts first, then (w, x) kc pairs in consumption order,
            # then the late-needed big weights
            nc.sync.dma_start(
                out=b_q_s, in_=b_q.rearrange("(m p) -> p m", p=128)
            )
            nc.sync.dma_start(out=tri_s, in_=tri16[:, :])
            nc.sync.dma_start(out=sel_s, in_=sel2[:, :])
            w_v_r = w_v.rearrange("(kc p) n -> p kc n", p=128)
            for kc in range(NKC):
                nc.sync.dma_start(
                    out=wqk_s[:, kc, 0:512], in_=w_qk_r[:, kc, 0:512]
                )
                nc.sync.dma_start(
                    out=xT_s[:, kc, 0:512], in_=xT_r[:, kc, 0:512]
                )
                nc.sync.dma_start(
                    out=wqk_s[:, kc, 512:1024], in_=w_qk_r[:, kc, 512:1024]
                )
                if kc == 5:
                    nc.sync.dma_start(
                        out=wv_s[:, 0:4, :], in_=w_v_r[:, 0:4, :]
                    )
            nc.sync.dma_start(out=wv_s[:, 4:8, :], in_=w_v_r[:, 4:8, :])
            for kc in range(NKC):
                nc.sync.dma_start(
                    out=xT_s[:, kc, 512:T], in_=xT_r[:, kc, 512:T]
                )
            for kcp in range(4):
                nc.sync.dma_start(
                    out=wpr_s[:, kcp, :], in_=w_pr_r[:, kcp, :]
                )

        def emit_consts():
            nc.vector.memset(ones_s, 1.0)
            nc.vector.memset(rr_s, 0.0)
            for n in range(NJ):
                nc.vector.tensor_copy(
                    vc[n][:, :, :, HD : HD + 1],
                    ones_s.rearrange("p (a b) -> p a b", a=4),
                )

        qts = {}

        def boot_phase1():
            """QKV for chunk 0, kc-outer over 8 simultaneous PSUM banks."""
            qt = qpool.tile([128, 4, 512], F16, tag="qt", name="qt0")
            qts[0] = qt
            ps_a = pshare.tile([128, 512], F32, tag="ps", name="bps0")
            ps_b = pshare.tile([128, 512], F32, tag="ps", name="bps1")
            st_a = psum_s.tile([128, 2, 512], F32, tag="sab", name="bst0")
            st_b = psum_s.tile([128, 2, 512], F32, tag="sab", name="bst1")
            o_a = psum_o.tile([128, 512], F32, tag="out0", name="bo0")
            o_b = psum_o.tile([128, 512], F32, tag="out1", name="bo1")
            accs = [ps_a, ps_b, st_a[:, 0, :], st_a[:, 1, :],
                    st_b[:, 0, :], st_b[:, 1, :], o_a, o_b]
            for kc in range(NKC):
                for m in range(8):
                    nc.tensor.matmul(
                        accs[m],
                        lhsT=wqk_s[:, kc, m * 128 : (m + 1) * 128],
                        rhs=xT_s[:, kc, 0:512],
                        start=(kc == 0), stop=(kc == NKC - 1),
                    )
            for m in range(4):
                nc.vector.tensor_scalar_add(
                    qt[:, m, :], accs[m], b_q_s[:, m : m + 1]
                )
            for m in range(4, 8):
                nc.vector.tensor_copy(ktc[0][:, m - 4, :], accs[m])
            for mv in range(4):
                for kc in range(NKC):
                    nc.tensor.matmul(
                        accs[mv],
                        lhsT=xT_s[:, kc, mv * 128 : (mv + 1) * 128],
                        rhs=wv_s[:, kc, :],
                        start=(kc == 0), stop=(kc == NKC - 1),
                    )
                nc.vector.tensor_copy(
                    vc[0][:, mv, :, 0:HD],
                    accs[mv].rearrange("p (h d) -> p h d", h=HPG),
                )

        def phase1_steps(n):
            """QKV projections for token chunk n; yields per PSUM group."""
            lo = n * 512
            qt = qpool.tile([128, 4, 512], F16, tag="qt", name=f"qt{n}")
            qts[n] = qt
            for m in range(8):
                ps = pshare.tile([128, 512], F32, tag="ps", name="ps")
                for kc in range(NKC):
                    nc.tensor.matmul(
                        ps,
                        lhsT=wqk_s[:, kc, m * 128 : (m + 1) * 128],
                        rhs=xT_s[:, kc, lo : lo + 512],
                        start=(kc == 0), stop=(kc == NKC - 1),
                    )
                if m < 4:
                    nc.vector.tensor_scalar_add(
                        qt[:, m, :], ps, b_q_s[:, m : m + 1]
                    )
                elif m % 2 == 0:
                    nc.vector.tensor_copy(ktc[n][:, m - 4, :], ps)
                else:
                    nc.scalar.copy(ktc[n][:, m - 4, :], ps)
                yield
            for mv in range(4):
                ps = pshare.tile([128, GC], F32, tag="ps", name="ps")
                for kc in range(NKC):
                    nc.tensor.matmul(
                        ps,
                        lhsT=xT_s[:, kc, lo + mv * 128 : lo + (mv + 1) * 128],
                        rhs=wv_s[:, kc, :],
                        start=(kc == 0), stop=(kc == NKC - 1),
                    )
                nc.vector.tensor_copy(
                    vc[n][:, mv, :, 0:HD],
                    ps.rearrange("p (h d) -> p h d", h=HPG),
                )
                yield

        prhss = {}

        def attention_steps(j):
            """Causal attention for q-chunk j; normalized OUT^T into prhs."""
            qt = qts[j]
            prhs = prhs_pool.tile([128, 4, 512], F16, tag="prhs",
                                  name=f"prhs{j}")
            prhss[j] = prhs
            nkc = 4 * j + 4
            for t in range(4):  # head pairs (2t, 2t+1)
                outs = [
                    psum_o.tile([128, 512], F32, tag=f"out{ab}",
                                name=f"out{ab}")
                    for ab in range(2)
                ]
                for kc in range(nkc):
                    c = kc - 4 * j  # >=0 -> diagonal band tile
                    lo = 128 * c if c > 0 else 0
                    st = psum_s.tile([128, 2, 512], F32, tag="sab", name="st")
                    for ab in range(2):
                        part = slice(ab * 64, ab * 64 + 64)
                        nc.tensor.matmul(
                            st[:, ab, lo:],
                            lhsT=ktc[kc // 4][part, t,
                                              (kc % 4) * 128 : (kc % 4 + 1) * 128],
                            rhs=qt[part, t, lo:],
                            start=True, stop=True,
                            tile_position=(ab * 64, 0),
                        )
                    pt = ppool.tile([128, 2, 512], F16, tag="pab", name="pt")
                    nc.scalar.activation(pt[:, :, lo:], st[:, :, lo:], AFT.Exp)
                    if c >= 0:
                        nc.vector.tensor_mul(
                            pt[:, :, lo : lo + 128],
                            pt[:, :, lo : lo + 128],
                            bcast(tri_s[:, :], 2, 1),
                        )
                    for ab in range(2):
                        nc.tensor.matmul(
                            outs[ab][0 : HD + 1, lo:],
                            lhsT=vc[kc // 4][:, kc % 4, 2 * t + ab, :],
                            rhs=pt[:, ab, lo:],
                            start=(kc == 0), stop=(kc == nkc - 1),
                        )
                    yield
                # normalize this pair: rank-2 PE broadcast of r from row 64,
                # then 1/r on the broadcast and PSUM-side multiplies
                nc.vector.tensor_copy(
                    rr_s[0:1, :], outs[0][HD : HD + 1, :]
                )
                nc.scalar.copy(
                    rr_s[32:33, :], outs[1][HD : HD + 1, :]
                )
                rbp = psum_s.tile([128, 2, 512], F32, tag="sab", name="rbp")
                nc.tensor.matmul(
                    rbp[:, 0, :], lhsT=sel_s, rhs=rr_s,
                    start=True, stop=True,
                )
                rb = rpool.tile([128, 512], F32, tag="rb", name="rb")
                nc.vector.reciprocal_approx_fast(out=rb, in_=rbp[:, 0, :])
                for ab in range(2):
                    part = slice(ab * 64, ab * 64 + 64)
                    nc.vector.tensor_mul(
                        prhs[part, t, :],
                        outs[ab][0:HD, :],
                        rb[part, :],
                    )
                if j == NJ - 1:
                    # keep the PE HAM-warm through the normalization chain so
                    # the final projection matmuls run at full clock
                    nwarm = 6 if t == 3 else 2
                    warm = psum_s.tile([128, 2, 512], F32, tag="sab",
                                       name="warm")
                    for w in range(nwarm):
                        nc.tensor.matmul(
                            warm[:, 0, :],
                            lhsT=wqk_s[:, w, 0:128],
                            rhs=wqk_s[:, w, 512 : 512 + 512],
                            start=True, stop=True,
                        )
                    nc.vector.tensor_copy(ones_s[0:1, 0:1], warm[0:1, 0, 0:1])
                yield

        def proj_steps(j):
            """Output projection for q-chunk j; yields per mp group."""
            prhs = prhss[j]
            for mp in range(8):
                ps = pshare.tile([128, 512], F32, tag="ps", name="ps")
                for kcp in range(4):
                    nc.tensor.matmul(
                        ps,
                        lhsT=wpr_s[:, kcp, mp * 128 : (mp + 1) * 128],
                        rhs=prhs[:, kcp, :],
                        start=(kcp == 0), stop=(kcp == 3),
                    )
                yb = ypool.tile([128, 512], F16, tag="yb", name="yb")
                rows = slice(mp * 128, (mp + 1) * 128)
                cols = slice(j * 512, (j + 1) * 512)
                if j == NJ - 1 and mp == 7:
                    # final group: split copy across DVE+ACT and halve the
                    # stores to shorten the end-of-kernel drain path
                    nc.vector.tensor_copy(yb[:, 0:256], ps[:, 0:256])
                    nc.scalar.copy(yb[:, 256:512], ps[:, 256:512])
                    nc.sync.dma_start(
                        out=yT[rows, j * 512 : j * 512 + 256],
                        in_=yb[:, 0:256],
                    )
                    nc.sync.dma_start(
                        out=yT[rows, j * 512 + 256 : (j + 1) * 512],
                        in_=yb[:, 256:512],
                    )
                else:
                    nc.vector.tensor_copy(yb, ps)
                    nc.sync.dma_start(out=yT[rows, cols], in_=yb)
                yield

        emit_input_dmas()
        emit_consts()
        boot_phase1()
        # Fill scheduling: QKV(chunk j+1) + proj(chunk j-1) groups interleave
        # into the PE gaps of ACT-paced attention(j), paced evenly.
        for j in range(NJ):
            fillers = []
            if j >= 1:
                fillers.append(proj_steps(j - 1))
            if j < NJ - 1:
                fillers.append(phase1_steps(j + 1))
            n_att = 4 * (4 * j + 5)
            n_fill = 8 * (j >= 1) + 12 * (j < NJ - 1)
            acc = 0.5
            fi = 0
            for _ in attention_steps(j):
                acc += n_fill / n_att
                while acc >= 1.0 and fillers:
                    gen = fillers[fi % len(fillers)]
                    try:
                        next(gen)
                        fi += 1
                        acc -= 1.0
                    except StopIteration:
                        fillers.remove(gen)
            for gen in fillers:
                for _ in gen:
                    pass
        for _ in proj_steps(NJ - 1):
            pass
    return nc


def round_fp32r(a):
    """Convert to fp16 for the PE fast path (full 2.4 GHz streaming rate)."""
    return np.ascontiguousarray(a, dtype=np.float16)


def make_core_inputs(x, w_attn, b_attn, w_proj):
    """Per-core input dicts; core index = batch*2 + head_group."""
    scale = np.float32(1.0 / np.sqrt(HD))
    tri = (
        np.arange(128)[:, None] <= np.arange(128)[None, :]
    ).astype(np.float16)
    sel = np.zeros((64, 128), dtype=np.float16)
    sel[0, 0:64] = 1.0
    sel[32, 64:128] = 1.0
    in_maps = []
    for b in range(NB):
        xTb = round_fp32r(x[b].T)
        for g in range(2):
            cols = slice(g * GC, (g + 1) * GC)
            w_qk = np.concatenate(
                [w_attn[:, cols] * scale, w_attn[:, C:][:, cols]], axis=1
            ).astype(np.float32)
            in_maps.append(
                {
                    "xT": xTb,
                    "w_qk": round_fp32r(w_qk),
                    "w_v": round_fp32r(w_attn[:, 2 * C :][:, cols]),
                    "b_q": np.ascontiguousarray(
                        b_attn[cols] * scale, dtype=np.float32
                    ),
                    "w_pr": round_fp32r(w_proj[cols, :]),
                    "tri16": tri,
                    "sel2": sel,
                }
            )
    return in_maps


_NC_CACHE = None


def get_nc():
    global _NC_CACHE
    if _NC_CACHE is None:
        nc = bacc.Bacc("TRN2", target_bir_lowering=False, debug=False)
        build_kernel(nc)
        nc.compile()
        _NC_CACHE = nc
    return _NC_CACHE


def kernel(x, w_attn, b_attn, w_proj, b_proj, _want_trace=False):
    x = np.asarray(x, dtype=np.float32)
    w_attn = np.asarray(w_attn, dtype=np.float32)
    b_attn = np.asarray(b_attn, dtype=np.float32)
    w_proj = np.asarray(w_proj, dtype=np.float32)
    b_proj = np.asarray(b_proj, dtype=np.float32)

    nc = get_nc()
    in_maps = make_core_inputs(x, w_attn, b_attn, w_proj)
    res = run_bass_kernel_spmd(
        nc, in_maps, list(range(8)), trace=_want_trace
    )
    bias_total = (b_attn[2 * C :] @ w_proj + b_proj).astype(np.float32)
    out = np.empty((NB, T, C), np.float32)
    for b in range(NB):
        out[b] = (
            res.results[2 * b]["yT"].T.astype(np.float32)
            + res.results[2 * b + 1]["yT"].T.astype(np.float32)
            + bias_total[None, :]
        )
    if _want_trace:
        return out, res
    return out


# revision 49
# speedup vs baseline: 1.0156x; 1.0156x over previous
"""Causal self-attention Trainium2 Bass kernel (8-core SPMD).

Problem: x[4,2048,1024] -> CausalSelfAttention(n_head=16) -> [4,2048,1024], f32.

Sharding: 8 cores = 4 batches x 2 head-groups (8 heads each). No collectives:
each core computes its head-group's partial output projection; the host sums
the two partials per batch and adds the folded bias.

Per-core dataflow (all matmuls fp16-in/f32-psum, transpose-free):
  All inputs (xT, w_qk, w_v, w_pr) are loaded ONCE into SBUF with large
  descriptors; steady state does no input DMA.
  boot:    QKV for token chunk 0 with kc-outer accumulation across 8 PSUM
           banks so the PE starts as soon as the first (w, x) slices land.
  phase 1: qT = (w_q*s).T @ xT + b_q*s   [512,2048]  feature-major
           kT = w_k.T @ xT               [512,2048]  (b_k dropped: softmax
                                                      row-shift invariance)
           V  = xT.T @ w_v               [2048,512]  token-major, augmented
                                                     with a ones column
  phase 2: per (q-chunk j, head pair): S^T = kT.T-slice @ qT-slice (k on
           partitions), additive causal mask on diagonal 128x128 squares,
           P = exp(S^T) (no max subtraction; scores are O(3)), PV^T
           accumulated over k-tiles with the ones column producing row sums r
           in row 64; normalize via a rank-2 PE broadcast of r (no DRAM
           bounce), reciprocal, and PSUM-side multiplies; then
           Y^T += w_pr.T @ OUT^T, with proj(j) interleaved into
           attention(j+1) as PE filler. The causal mask is applied
           multiplicatively on P (0/1 triangle) after exp.
Host folds b_v and b_proj into one constant vector: b_v @ w_proj + b_proj
(softmax rows sum to 1, so the V bias passes through exactly).
"""

import sys

if "/opt/trn_rl_repo" not in sys.path:
    sys.path.insert(0, "/opt/trn_rl_repo")

from contextlib import ExitStack

import numpy as np

import concourse.bass as bass
import concourse.tile as tile
from concourse import bacc, mybir
from concourse.bass_utils import run_bass_kernel_spmd

F32 = mybir.dt.float32
F16 = mybir.dt.float16
AFT = mybir.ActivationFunctionType

C = 1024          # n_embd
T = 2048          # seq len
NB = 4            # batch
NHEAD = 16
HD = 64           # head dim
HPG = 8           # heads per group (per core)
GC = HPG * HD     # 512 features per group
NKC = C // 128    # 8 contraction tiles over C
NJ = T // 512     # 4 q-chunks of 512


def bcast(ap, n, axis):
    """Insert a step-0 (broadcast) dim of size n at free-dim position axis."""
    steps = list(ap.ap)
    steps.insert(axis, [0, n])
    return bass.AP(tensor=ap.tensor, offset=ap.offset, ap=steps)


def build_kernel(nc: bass.Bass):
    xT = nc.dram_tensor("xT", [C, T], F16, kind="ExternalInput")
    w_qk = nc.dram_tensor("w_qk", [C, 2 * GC], F16, kind="ExternalInput")
    w_v = nc.dram_tensor("w_v", [C, GC], F16, kind="ExternalInput")
    b_q = nc.dram_tensor("b_q", [GC], F32, kind="ExternalInput")
    w_pr = nc.dram_tensor("w_pr", [GC, C], F16, kind="ExternalInput")
    tri16 = nc.dram_tensor("tri16", [128, 128], F16, kind="ExternalInput")
    sel2 = nc.dram_tensor("sel2", [64, 128], F16, kind="ExternalInput")
    yT = nc.dram_tensor("yT", [C, T], F16, kind="ExternalOutput")

    xT_r = xT.rearrange("(kc p) t -> p kc t", p=128)
    w_qk_r = w_qk.rearrange("(kc p) c -> p kc c", p=128)
    w_pr_r = w_pr.rearrange("(kc p) c -> p kc c", p=128)

    with tile.TileContext(nc) as tc, ExitStack() as ctx:
        persist = ctx.enter_context(tc.tile_pool(name="persist", bufs=1))
        qpool = ctx.enter_context(tc.tile_pool(name="qpool", bufs=2))
        ppool = ctx.enter_context(tc.tile_pool(name="ppool", bufs=4))
        rpool = ctx.enter_context(tc.tile_pool(name="rpool", bufs=2))
        prhs_pool = ctx.enter_context(tc.tile_pool(name="prhsp", bufs=2))
        ypool = ctx.enter_context(tc.tile_pool(name="ypool", bufs=3))
        pshare = ctx.enter_context(
            tc.tile_pool(name="pshare", bufs=2, space="PSUM")
        )
        psum_s = ctx.enter_context(
            tc.tile_pool(name="psum_s", bufs=2, space="PSUM")
        )
        psum_o = ctx.enter_context(
            tc.tile_pool(name="psum_o", bufs=1, space="PSUM")
        )

        # per-chunk K^T and V tiles (separate tiles so chunk n+1 writes
        # never alias chunk <=n reads during cross-phase interleaving)
        ktc = [persist.tile([128, 4, 512], F16, tag=f"kt{n}", name=f"kt{n}")
               for n in range(NJ)]
        vc = [persist.tile([128, 4, HPG, HD + 1], F16, tag=f"v{n}",
                           name=f"v{n}") for n in range(NJ)]
        xT_s = persist.tile([128, NKC, T], F16, tag="xs")
        wqk_s = persist.tile([128, NKC, 2 * GC], F16, tag="wqk")
        wv_s = persist.tile([128, NKC, GC], F16, tag="wv")
        wpr_s = persist.tile([128, 4, C], F16, tag="wpr")
        b_q_s = persist.tile([128, 4], F32, tag="bq")
        tri_s = persist.tile([128, 128], F16, tag="tri")
        ones_s = persist.tile([128, 4 * HPG], F32, tag="ones")
        sel_s = persist.tile([64, 128], F16, tag="sel")
        rr_s = persist.tile([64, 512], F16, tag="rr")

        def emit_input_dmas():
            # tiny consts first, then (w, x) kc pairs in consumption order,
            # then the late-needed big weights
            nc.sync.dma_start(
                out=b_q_s, in_=b_q.rearrange("(m p) -> p m", p=128)
            )
            nc.sync.dma_start(out=tri_s, in_=tri16[:, :])
            nc.sync.dma_start(out=sel_s, in_=sel2[:, :])
            w_v_r = w_v.rearrange("(kc p) n -> p kc n", p=128)
            for kc in range(NKC):
                nc.sync.dma_start(
                    out=wqk_s[:, kc, 0:512], in_=w_qk_r[:, kc, 0:512]
                )
                nc.sync.dma_start(
                    out=xT_s[:, kc, 0:512], in_=xT_r[:, kc, 0:512]
                )
                nc.sync.dma_start(
                    out=wqk_s[:, kc, 512:1024], in_=w_qk_r[:, kc, 512:1024]
                )
                if kc == 4:
                    nc.sync.dma_start(
                        out=wv_s[:, 0:4, :], in_=w_v_r[:, 0:4, :]
                    )
                if kc == 6:
                    nc.sync.dma_start(
                        out=wv_s[:, 4:8, :], in_=w_v_r[:, 4:8, :]
                    )
            for kc in range(NKC):
                nc.sync.dma_start(
                    out=xT_s[:, kc, 512:T], in_=xT_r[:, kc, 512:T]
                )
            for kcp in range(4):
                nc.sync.dma_start(
                    out=wpr_s[:, kcp, :], in_=w_pr_r[:, kcp, :]
                )

        def emit_consts():
            nc.vector.memset(ones_s, 1.0)
            nc.vector.memset(rr_s, 0.0)
            for n in range(NJ):
                nc.vector.tensor_copy(
                    vc[n][:, :, :, HD : HD + 1],
                    ones_s.rearrange("p (a b) -> p a b", a=4),
                )

        qts = {}

        def boot_phase1():
            """QKV for chunk 0, kc-outer over 8 simultaneous PSUM banks."""
            qt = qpool.tile([128, 4, 512], F16, tag="qt", name="qt0")
            qts[0] = qt
            ps_a = pshare.tile([128, 512], F32, tag="ps", name="bps0")
            ps_b = pshare.tile([128, 512], F32, tag="ps", name="bps1")
            st_a = psum_s.tile([128, 2, 512], F32, tag="sab", name="bst0")
            st_b = psum_s.tile([128, 2, 512], F32, tag="sab", name="bst1")
            o_a = psum_o.tile([128, 512], F32, tag="out0", name="bo0")
            o_b = psum_o.tile([128, 512], F32, tag="out1", name="bo1")
            accs = [ps_a, ps_b, st_a[:, 0, :], st_a[:, 1, :],
                    st_b[:, 0, :], st_b[:, 1, :], o_a, o_b]
            for kc in range(NKC):
                for m in range(8):
                    nc.tensor.matmul(
                        accs[m],
                        lhsT=wqk_s[:, kc, m * 128 : (m + 1) * 128],
                        rhs=xT_s[:, kc, 0:512],
                        start=(kc == 0), stop=(kc == NKC - 1),
                    )
            for m in range(4):
                nc.vector.tensor_scalar_add(
                    qt[:, m, :], accs[m], b_q_s[:, m : m + 1]
                )
            for m in range(4, 8):
                nc.vector.tensor_copy(ktc[0][:, m - 4, :], accs[m])
            for mv in range(4):
                for kc in range(NKC):
                    nc.tensor.matmul(
                        accs[mv],
                        lhsT=xT_s[:, kc, mv * 128 : (mv + 1) * 128],
                        rhs=wv_s[:, kc, :],
                        start=(kc == 0), stop=(kc == NKC - 1),
                    )
                nc.vector.tensor_copy(
                    vc[0][:, mv, :, 0:HD],
                    accs[mv].rearrange("p (h d) -> p h d", h=HPG),
                )

        def phase1_steps(n):
            """QKV projections for token chunk n; yields per PSUM group."""
            lo = n * 512
            qt = qpool.tile([128, 4, 512], F16, tag="qt", name=f"qt{n}")
            qts[n] = qt
            for m in range(8):
                ps = pshare.tile([128, 512], F32, tag="ps", name="ps")
                for kc in range(NKC):
                    nc.tensor.matmul(
                        ps,
                        lhsT=wqk_s[:, kc, m * 128 : (m + 1) * 128],
                        rhs=xT_s[:, kc, lo : lo + 512],
                        start=(kc == 0), stop=(kc == NKC - 1),
                    )
                if m < 4:
                    nc.vector.tensor_scalar_add(
                        qt[:, m, :], ps, b_q_s[:, m : m + 1]
                    )
                elif m % 2 == 0:
                    nc.vector.tensor_copy(ktc[n][:, m - 4, :], ps)
                else:
                    nc.scalar.copy(ktc[n][:, m - 4, :], ps)
                yield
            for mv in range(4):
                ps = pshare.tile([128, GC], F32, tag="ps", name="ps")
                for kc in range(NKC):
                    nc.tensor.matmul(
                        ps,
                        lhsT=xT_s[:, kc, lo + mv * 128 : lo + (mv + 1) * 128],
                        rhs=wv_s[:, kc, :],
                        start=(kc == 0), stop=(kc == NKC - 1),
                    )
                nc.vector.tensor_copy(
                    vc[n][:, mv, :, 0:HD],
                    ps.rearrange("p (h d) -> p h d", h=HPG),
                )
                yield

        prhss = {}

        def attention_steps(j):
            """Causal attention for q-chunk j; normalized OUT^T into prhs."""
            qt = qts[j]
            prhs = prhs_pool.tile([128, 4, 512], F16, tag="prhs",
                                  name=f"prhs{j}")
            prhss[j] = prhs
            nkc = 4 * j + 4
            for t in range(4):  # head pairs (2t, 2t+1)
                outs = [
                    psum_o.tile([128, 512], F32, tag=f"out{ab}",
                                name=f"out{ab}")
                    for ab in range(2)
                ]
                for kc in range(nkc):
                    c = kc - 4 * j  # >=0 -> diagonal band tile
                    lo = 128 * c if c > 0 else 0
                    st = psum_s.tile([128, 2, 512], F32, tag="sab", name="st")
                    for ab in range(2):
                        part = slice(ab * 64, ab * 64 + 64)
                        nc.tensor.matmul(
                            st[:, ab, lo:],
                            lhsT=ktc[kc // 4][part, t,
                                              (kc % 4) * 128 : (kc % 4 + 1) * 128],
                            rhs=qt[part, t, lo:],
                            start=True, stop=True,
                            tile_position=(ab * 64, 0),
                        )
                    pt = ppool.tile([128, 2, 512], F16, tag="pab", name="pt")
                    nc.scalar.activation(pt[:, :, lo:], st[:, :, lo:], AFT.Exp)
                    if c >= 0:
                        nc.vector.tensor_mul(
                            pt[:, :, lo : lo + 128],
                            pt[:, :, lo : lo + 128],
                            bcast(tri_s[:, :], 2, 1),
                        )
                    for ab in range(2):
                        nc.tensor.matmul(
                            outs[ab][0 : HD + 1, lo:],
                            lhsT=vc[kc // 4][:, kc % 4, 2 * t + ab, :],
                            rhs=pt[:, ab, lo:],
                            start=(kc == 0), stop=(kc == nkc - 1),
                        )
                    yield
                # normalize this pair: rank-2 PE broadcast of r from row 64,
                # then 1/r on the broadcast and PSUM-side multiplies
                nc.vector.tensor_copy(
                    rr_s[0:1, :], outs[0][HD : HD + 1, :]
                )
                nc.scalar.copy(
                    rr_s[32:33, :], outs[1][HD : HD + 1, :]
                )
                rbp = psum_s.tile([128, 2, 512], F32, tag="sab", name="rbp")
                nc.tensor.matmul(
                    rbp[:, 0, :], lhsT=sel_s, rhs=rr_s,
                    start=True, stop=True,
                )
                rb = rpool.tile([128, 512], F32, tag="rb", name="rb")
                nc.vector.reciprocal_approx_fast(out=rb, in_=rbp[:, 0, :])
                for ab in range(2):
                    part = slice(ab * 64, ab * 64 + 64)
                    nc.vector.tensor_mul(
                        prhs[part, t, :],
                        outs[ab][0:HD, :],
                        rb[part, :],
                    )
                if j == NJ - 1:
                    # keep the PE HAM-warm through the normalization chain so
                    # the final projection matmuls run at full clock
                    nwarm = 6 if t == 3 else 2
                    warm = psum_s.tile([128, 2, 512], F32, tag="sab",
                                       name="warm")
                    for w in range(nwarm):
                        nc.tensor.matmul(
                            warm[:, 0, :],
                            lhsT=wqk_s[:, w, 0:128],
                            rhs=wqk_s[:, w, 512 : 512 + 512],
                            start=True, stop=True,
                        )
                    nc.vector.tensor_copy(ones_s[0:1, 0:1], warm[0:1, 0, 0:1])
                yield

        def proj_steps(j):
            """Output projection for q-chunk j; yields per mp group."""
            prhs = prhss[j]
            for mp in range(8):
                ps = pshare.tile([128, 512], F32, tag="ps", name="ps")
                for kcp in range(4):
                    nc.tensor.matmul(
                        ps,
                        lhsT=wpr_s[:, kcp, mp * 128 : (mp + 1) * 128],
                        rhs=prhs[:, kcp, :],
                        start=(kcp == 0), stop=(kcp == 3),
                    )
                yb = ypool.tile([128, 512], F16, tag="yb", name="yb")
                rows = slice(mp * 128, (mp + 1) * 128)
                cols = slice(j * 512, (j + 1) * 512)
                if j == NJ - 1 and mp == 7:
                    # final group: split copy across DVE+ACT and halve the
                    # stores to shorten the end-of-kernel drain path
                    nc.vector.tensor_copy(yb[:, 0:256], ps[:, 0:256])
                    nc.scalar.copy(yb[:, 256:512], ps[:, 256:512])
                    nc.sync.dma_start(
                        out=yT[rows, j * 512 : j * 512 + 256],
                        in_=yb[:, 0:256],
                    )
                    nc.sync.dma_start(
                        out=yT[rows, j * 512 + 256 : (j + 1) * 512],
                        in_=yb[:, 256:512],
                    )
                else:
                    nc.vector.tensor_copy(yb, ps)
                    nc.sync.dma_start(out=yT[rows, cols], in_=yb)
                yield

        emit_input_dmas()
        emit_consts()
        boot_phase1()
        # Fill scheduling: QKV(chunk j+1) + proj(chunk j-1) groups interleave
        # into the PE gaps of ACT-paced attention(j), paced evenly.
        for j in range(NJ):
            fillers = []
            if j >= 1:
                fillers.append(proj_steps(j - 1))
            if j < NJ - 1:
                fillers.append(phase1_steps(j + 1))
            n_att = 4 * (4 * j + 5)
            n_fill = 8 * (j >= 1) + 12 * (j < NJ - 1)
            acc = 0.5
            fi = 0
            for _ in attention_steps(j):
                acc += n_fill / n_att
                while acc >= 1.0 and fillers:
                    gen = fillers[fi % len(fillers)]
                    try:
                        next(gen)
                        fi += 1
                        acc -= 1.0
                    except StopIteration:
                        fillers.remove(gen)
            for gen in fillers:
                for _ in gen:
                    pass
        for _ in proj_steps(NJ - 1):
            pass
    return nc


def round_fp32r(a):
    """Convert to fp16 for the PE fast path (full 2.4 GHz streaming rate)."""
    return np.ascontiguousarray(a, dtype=np.float16)


def make_core_inputs(x, w_attn, b_attn, w_proj):
    """Per-core input dicts; core index = batch*2 + head_group."""
    scale = np.float32(1.0 / np.sqrt(HD))
    tri = (
        np.arange(128)[:, None] <= np.arange(128)[None, :]
    ).astype(np.float16)
    sel = np.zeros((64, 128), dtype=np.float16)
    sel[0, 0:64] = 1.0
    sel[32, 64:128] = 1.0
    in_maps = []
    for b in range(NB):
        xTb = round_fp32r(x[b].T)
        for g in range(2):
            cols = slice(g * GC, (g + 1) * GC)
            w_qk = np.concatenate(
                [w_attn[:, cols] * scale, w_attn[:, C:][:, cols]], axis=1
            ).astype(np.float32)
            in_maps.append(
                {
                    "xT": xTb,
                    "w_qk": round_fp32r(w_qk),
                    "w_v": round_fp32r(w_attn[:, 2 * C :][:, cols]),
                    "b_q": np.ascontiguousarray(
                        b_attn[cols] * scale, dtype=np.float32
                    ),
                    "w_pr": round_fp32r(w_proj[cols, :]),
                    "tri16": tri,
                    "sel2": sel,
                }
            )
    return in_maps


_NC_CACHE = None


def get_nc():
    global _NC_CACHE
    if _NC_CACHE is None:
        nc = bacc.Bacc("TRN2", target_bir_lowering=False, debug=False)
        build_kernel(nc)
        nc.compile()
        _NC_CACHE = nc
    return _NC_CACHE


def kernel(x, w_attn, b_attn, w_proj, b_proj, _want_trace=False):
    x = np.asarray(x, dtype=np.float32)
    w_attn = np.asarray(w_attn, dtype=np.float32)
    b_attn = np.asarray(b_attn, dtype=np.float32)
    w_proj = np.asarray(w_proj, dtype=np.float32)
    b_proj = np.asarray(b_proj, dtype=np.float32)

    nc = get_nc()
    in_maps = make_core_inputs(x, w_attn, b_attn, w_proj)
    res = run_bass_kernel_spmd(
        nc, in_maps, list(range(8)), trace=_want_trace
    )
    bias_total = (b_attn[2 * C :] @ w_proj + b_proj).astype(np.float32)
    out = np.empty((NB, T, C), np.float32)
    for b in range(NB):
        out[b] = (
            res.results[2 * b]["yT"].T.astype(np.float32)
            + res.results[2 * b + 1]["yT"].T.astype(np.float32)
            + bias_total[None, :]
        )
    if _want_trace:
        return out, res
    return out


# revision 50
# speedup vs baseline: 1.0210x; 1.0053x over previous
"""Causal self-attention Trainium2 Bass kernel (8-core SPMD).

Problem: x[4,2048,1024] -> CausalSelfAttention(n_head=16) -> [4,2048,1024], f32.

Sharding: 8 cores = 4 batches x 2 head-groups (8 heads each). No collectives:
each core computes its head-group's partial output projection; the host sums
the two partials per batch and adds the folded bias.

Per-core dataflow (all matmuls fp16-in/f32-psum, transpose-free):
  All inputs (xT, w_qk, w_v, w_pr) are loaded ONCE into SBUF with large
  descriptors; steady state does no input DMA.
  boot:    QKV for token chunk 0 with kc-outer accumulation across 8 PSUM
           banks so the PE starts as soon as the first (w, x) slices land.
  phase 1: qT = (w_q*s).T @ xT + b_q*s   [512,2048]  feature-major
           kT = w_k.T @ xT               [512,2048]  (b_k dropped: softmax
                                                      row-shift invariance)
           V  = xT.T @ w_v               [2048,512]  token-major, augmented
                                                     with a ones column
  phase 2: per (q-chunk j, head pair): S^T = kT.T-slice @ qT-slice (k on
           partitions), additive causal mask on diagonal 128x128 squares,
           P = exp(S^T) (no max subtraction; scores are O(3)), PV^T
           accumulated over k-tiles with the ones column producing row sums r
           in row 64; normalize via a rank-2 PE broadcast of r (no DRAM
           bounce), reciprocal, and PSUM-side multiplies; then
           Y^T += w_pr.T @ OUT^T, with proj(j) interleaved into
           attention(j+1) as PE filler. The causal mask is applied
           multiplicatively on P (0/1 triangle) after exp.
Host folds b_v and b_proj into one constant vector: b_v @ w_proj + b_proj
(softmax rows sum to 1, so the V bias passes through exactly).
"""

import sys

if "/opt/trn_rl_repo" not in sys.path:
    sys.path.insert(0, "/opt/trn_rl_repo")

from contextlib import ExitStack

import numpy as np

import concourse.bass as bass
import concourse.tile as tile
from concourse import bacc, mybir
from concourse.bass_utils import run_bass_kernel_spmd

F32 = mybir.dt.float32
F16 = mybir.dt.float16
AFT = mybir.ActivationFunctionType

C = 1024          # n_embd
T = 2048          # seq len
NB = 4            # batch
NHEAD = 16
HD = 64           # head dim
HPG = 8           # heads per group (per core)
GC = HPG * HD     # 512 features per group
NKC = C // 128    # 8 contraction tiles over C
NJ = T // 512     # 4 q-chunks of 512


def bcast(ap, n, axis):
    """Insert a step-0 (broadcast) dim of size n at free-dim position axis."""
    steps = list(ap.ap)
    steps.insert(axis, [0, n])
    return bass.AP(tensor=ap.tensor, offset=ap.offset, ap=steps)


def build_kernel(nc: bass.Bass):
    xT = nc.dram_tensor("xT", [C, T], F16, kind="ExternalInput")
    w_qk = nc.dram_tensor("w_qk", [C, 2 * GC], F16, kind="ExternalInput")
    w_v = nc.dram_tensor("w_v", [C, GC], F16, kind="ExternalInput")
    b_q = nc.dram_tensor("b_q", [GC], F32, kind="ExternalInput")
    w_pr = nc.dram_tensor("w_pr", [GC, C], F16, kind="ExternalInput")
    tri16 = nc.dram_tensor("tri16", [128, 128], F16, kind="ExternalInput")
    sel2 = nc.dram_tensor("sel2", [64, 128], F16, kind="ExternalInput")
    yT = nc.dram_tensor("yT", [C, T], F16, kind="ExternalOutput")

    xT_r = xT.rearrange("(kc p) t -> p kc t", p=128)
    w_qk_r = w_qk.rearrange("(kc p) c -> p kc c", p=128)
    w_pr_r = w_pr.rearrange("(kc p) c -> p kc c", p=128)

    with tile.TileContext(nc) as tc, ExitStack() as ctx:
        persist = ctx.enter_context(tc.tile_pool(name="persist", bufs=1))
        qpool = ctx.enter_context(tc.tile_pool(name="qpool", bufs=2))
        ppool = ctx.enter_context(tc.tile_pool(name="ppool", bufs=4))
        rpool = ctx.enter_context(tc.tile_pool(name="rpool", bufs=2))
        prhs_pool = ctx.enter_context(tc.tile_pool(name="prhsp", bufs=2))
        ypool = ctx.enter_context(tc.tile_pool(name="ypool", bufs=3))
        pshare = ctx.enter_context(
            tc.tile_pool(name="pshare", bufs=2, space="PSUM")
        )
        psum_s = ctx.enter_context(
            tc.tile_pool(name="psum_s", bufs=2, space="PSUM")
        )
        psum_o = ctx.enter_context(
            tc.tile_pool(name="psum_o", bufs=1, space="PSUM")
        )

        # per-chunk K^T and V tiles (separate tiles so chunk n+1 writes
        # never alias chunk <=n reads during cross-phase interleaving)
        ktc = [persist.tile([128, 4, 512], F16, tag=f"kt{n}", name=f"kt{n}")
               for n in range(NJ)]
        vc = [persist.tile([128, 4, HPG, HD + 1], F16, tag=f"v{n}",
                           name=f"v{n}") for n in range(NJ)]
        xT_s = persist.tile([128, NKC, T], F16, tag="xs")
        wqk_s = persist.tile([128, NKC, 2 * GC], F16, tag="wqk")
        wv_s = persist.tile([128, NKC, GC], F16, tag="wv")
        wpr_s = persist.tile([128, 4, C], F16, tag="wpr")
        b_q_s = persist.tile([128, 4], F32, tag="bq")
        tri_s = persist.tile([128, 128], F16, tag="tri")
        ones_s = persist.tile([128, 4 * HPG], F32, tag="ones")
        sel_s = persist.tile([64, 128], F16, tag="sel")
        rr_s = persist.tile([64, 512], F16, tag="rr")

        def emit_input_dmas():
            # tiny consts first, then (w, x) kc pairs in consumption order,
            # then the late-needed big weights
            nc.sync.dma_start(
                out=b_q_s, in_=b_q.rearrange("(m p) -> p m", p=128)
            )
            nc.sync.dma_start(out=tri_s, in_=tri16[:, :])
            nc.sync.dma_start(out=sel_s, in_=sel2[:, :])
            w_v_r = w_v.rearrange("(kc p) n -> p kc n", p=128)
            for kc in range(NKC):
                nc.sync.dma_start(
                    out=wqk_s[:, kc, 0:512], in_=w_qk_r[:, kc, 0:512]
                )
                nc.sync.dma_start(
                    out=xT_s[:, kc, 0:512], in_=xT_r[:, kc, 0:512]
                )
                nc.sync.dma_start(
                    out=wqk_s[:, kc, 512:1024], in_=w_qk_r[:, kc, 512:1024]
                )
                if kc == 4:
                    nc.sync.dma_start(
                        out=wv_s[:, 0:4, :], in_=w_v_r[:, 0:4, :]
                    )
                if kc == 6:
                    nc.sync.dma_start(
                        out=wv_s[:, 4:8, :], in_=w_v_r[:, 4:8, :]
                    )
            for kc in range(NKC):
                nc.sync.dma_start(
                    out=xT_s[:, kc, 512:T], in_=xT_r[:, kc, 512:T]
                )
            for kcp in range(4):
                nc.sync.dma_start(
                    out=wpr_s[:, kcp, :], in_=w_pr_r[:, kcp, :]
                )

        def emit_consts():
            nc.vector.memset(ones_s, 1.0)
            nc.vector.memset(rr_s, 0.0)
            for n in range(NJ):
                nc.vector.tensor_copy(
                    vc[n][:, :, :, HD : HD + 1],
                    ones_s.rearrange("p (a b) -> p a b", a=4),
                )

        qts = {}

        def boot_phase1():
            """QKV for chunk 0, kc-outer over 8 simultaneous PSUM banks."""
            qt = qpool.tile([128, 4, 512], F16, tag="qt", name="qt0")
            qts[0] = qt
            ps_a = pshare.tile([128, 512], F32, tag="ps", name="bps0")
            ps_b = pshare.tile([128, 512], F32, tag="ps", name="bps1")
            st_a = psum_s.tile([128, 2, 512], F32, tag="sab", name="bst0")
            st_b = psum_s.tile([128, 2, 512], F32, tag="sab", name="bst1")
            o_a = psum_o.tile([128, 512], F32, tag="out0", name="bo0")
            o_b = psum_o.tile([128, 512], F32, tag="out1", name="bo1")
            accs = [ps_a, ps_b, st_a[:, 0, :], st_a[:, 1, :],
                    st_b[:, 0, :], st_b[:, 1, :], o_a, o_b]
            for kc in range(NKC):
                for m in range(8):
                    nc.tensor.matmul(
                        accs[m],
                        lhsT=wqk_s[:, kc, m * 128 : (m + 1) * 128],
                        rhs=xT_s[:, kc, 0:512],
                        start=(kc == 0), stop=(kc == NKC - 1),
                    )
            for m in range(4):
                nc.vector.tensor_scalar_add(
                    qt[:, m, :], accs[m], b_q_s[:, m : m + 1]
                )
            for m in range(4, 8):
                nc.vector.tensor_copy(ktc[0][:, m - 4, :], accs[m])
            for mv in range(4):
                for kc in range(NKC):
                    nc.tensor.matmul(
                        accs[mv],
                        lhsT=xT_s[:, kc, mv * 128 : (mv + 1) * 128],
                        rhs=wv_s[:, kc, :],
                        start=(kc == 0), stop=(kc == NKC - 1),
                    )
                nc.vector.tensor_copy(
                    vc[0][:, mv, :, 0:HD],
                    accs[mv].rearrange("p (h d) -> p h d", h=HPG),
                )

        def phase1_steps(n):
            """QKV projections for token chunk n; yields per PSUM group."""
            lo = n * 512
            qt = qpool.tile([128, 4, 512], F16, tag="qt", name=f"qt{n}")
            qts[n] = qt
            for m in range(8):
                ps = pshare.tile([128, 512], F32, tag="ps", name="ps")
                for kc in range(NKC):
                    nc.tensor.matmul(
                        ps,
                        lhsT=wqk_s[:, kc, m * 128 : (m + 1) * 128],
                        rhs=xT_s[:, kc, lo : lo + 512],
                        start=(kc == 0), stop=(kc == NKC - 1),
                    )
                if m < 4:
                    nc.vector.tensor_scalar_add(
                        qt[:, m, :], ps, b_q_s[:, m : m + 1]
                    )
                elif m % 2 == 0:
                    nc.vector.tensor_copy(ktc[n][:, m - 4, :], ps)
                else:
                    nc.scalar.copy(ktc[n][:, m - 4, :], ps)
                yield
            for mv in range(4):
                ps = pshare.tile([128, GC], F32, tag="ps", name="ps")
                for kc in range(NKC):
                    nc.tensor.matmul(
                        ps,
                        lhsT=xT_s[:, kc, lo + mv * 128 : lo + (mv + 1) * 128],
                        rhs=wv_s[:, kc, :],
                        start=(kc == 0), stop=(kc == NKC - 1),
                    )
                nc.vector.tensor_copy(
                    vc[n][:, mv, :, 0:HD],
                    ps.rearrange("p (h d) -> p h d", h=HPG),
                )
                yield

        prhss = {}

        def attention_steps(j):
            """Causal attention for q-chunk j; normalized OUT^T into prhs."""
            qt = qts[j]
            prhs = prhs_pool.tile([128, 4, 512], F16, tag="prhs",
                                  name=f"prhs{j}")
            prhss[j] = prhs
            nkc = 4 * j + 4
            for t in range(4):  # head pairs (2t, 2t+1)
                outs = [
                    psum_o.tile([128, 512], F32, tag=f"out{ab}",
                                name=f"out{ab}")
                    for ab in range(2)
                ]
                for kc in range(nkc):
                    c = kc - 4 * j  # >=0 -> diagonal band tile
                    lo = 128 * c if c > 0 else 0
                    st = psum_s.tile([128, 2, 512], F32, tag="sab", name="st")
                    for ab in range(2):
                        part = slice(ab * 64, ab * 64 + 64)
                        nc.tensor.matmul(
                            st[:, ab, lo:],
                            lhsT=ktc[kc // 4][part, t,
                                              (kc % 4) * 128 : (kc % 4 + 1) * 128],
                            rhs=qt[part, t, lo:],
                            start=True, stop=True,
                            tile_position=(ab * 64, 0),
                        )
                    pt = ppool.tile([128, 2, 512], F16, tag="pab", name="pt")
                    nc.scalar.activation(pt[:, :, lo:], st[:, :, lo:], AFT.Exp)
                    if c >= 0:
                        nc.vector.tensor_mul(
                            pt[:, :, lo : lo + 128],
                            pt[:, :, lo : lo + 128],
                            bcast(tri_s[:, :], 2, 1),
                        )
                    for ab in range(2):
                        nc.tensor.matmul(
                            outs[ab][0 : HD + 1, lo:],
                            lhsT=vc[kc // 4][:, kc % 4, 2 * t + ab, :],
                            rhs=pt[:, ab, lo:],
                            start=(kc == 0), stop=(kc == nkc - 1),
                        )
                    yield
                # normalize this pair: rank-2 PE broadcast of r from row 64,
                # then 1/r on the broadcast and PSUM-side multiplies
                nc.vector.tensor_copy(
                    rr_s[0:1, :], outs[0][HD : HD + 1, :]
                )
                nc.scalar.copy(
                    rr_s[32:33, :], outs[1][HD : HD + 1, :]
                )
                rbp = psum_s.tile([128, 2, 512], F32, tag="sab", name="rbp")
                nc.tensor.matmul(
                    rbp[:, 0, :], lhsT=sel_s, rhs=rr_s,
                    start=True, stop=True,
                )
                rb = rpool.tile([128, 512], F32, tag="rb", name="rb")
                nc.vector.reciprocal_approx_fast(out=rb, in_=rbp[:, 0, :])
                for ab in range(2):
                    part = slice(ab * 64, ab * 64 + 64)
                    nc.vector.tensor_mul(
                        prhs[part, t, :],
                        outs[ab][0:HD, :],
                        rb[part, :],
                    )
                if j == NJ - 1 and t == 3:
                    # keep the PE HAM-warm through the final normalization
                    # chain so the last projection matmuls run at full clock
                    nwarm = 6
                    warm = psum_s.tile([128, 2, 512], F32, tag="sab",
                                       name="warm")
                    for w in range(nwarm):
                        nc.tensor.matmul(
                            warm[:, 0, :],
                            lhsT=wqk_s[:, w, 0:128],
                            rhs=wqk_s[:, w, 512 : 512 + 512],
                            start=True, stop=True,
                        )
                    nc.vector.tensor_copy(ones_s[0:1, 0:1], warm[0:1, 0, 0:1])
                yield

        def proj_steps(j):
            """Output projection for q-chunk j; yields per mp group."""
            prhs = prhss[j]
            for mp in range(8):
                ps = pshare.tile([128, 512], F32, tag="ps", name="ps")
                for kcp in range(4):
                    nc.tensor.matmul(
                        ps,
                        lhsT=wpr_s[:, kcp, mp * 128 : (mp + 1) * 128],
                        rhs=prhs[:, kcp, :],
                        start=(kcp == 0), stop=(kcp == 3),
                    )
                yb = ypool.tile([128, 512], F16, tag="yb", name="yb")
                rows = slice(mp * 128, (mp + 1) * 128)
                cols = slice(j * 512, (j + 1) * 512)
                if j == NJ - 1 and mp == 7:
                    # final group: split copy across DVE+ACT and halve the
                    # stores to shorten the end-of-kernel drain path
                    nc.vector.tensor_copy(yb[:, 0:256], ps[:, 0:256])
                    nc.scalar.copy(yb[:, 256:512], ps[:, 256:512])
                    nc.sync.dma_start(
                        out=yT[rows, j * 512 : j * 512 + 256],
                        in_=yb[:, 0:256],
                    )
                    nc.sync.dma_start(
                        out=yT[rows, j * 512 + 256 : (j + 1) * 512],
                        in_=yb[:, 256:512],
                    )
                else:
                    nc.vector.tensor_copy(yb, ps)
                    nc.sync.dma_start(out=yT[rows, cols], in_=yb)
                yield

        emit_input_dmas()
        emit_consts()
        boot_phase1()
        # Fill scheduling: QKV(chunk j+1) + proj(chunk j-1) groups interleave
        # into the PE gaps of ACT-paced attention(j), paced evenly.
        for j in range(NJ):
            fillers = []
            if j >= 1:
                fillers.append(proj_steps(j - 1))
            if j < NJ - 1:
                fillers.append(phase1_steps(j + 1))
            n_att = 4 * (4 * j + 5)
            n_fill = 8 * (j >= 1) + 12 * (j < NJ - 1)
            acc = 0.5
            fi = 0
            for _ in attention_steps(j):
                acc += n_fill / n_att
                while acc >= 1.0 and fillers:
                    gen = fillers[fi % len(fillers)]
                    try:
                        next(gen)
                        fi += 1
                        acc -= 1.0
                    except StopIteration:
                        fillers.remove(gen)
            for gen in fillers:
                for _ in gen:
                    pass
        for _ in proj_steps(NJ - 1):
            pass
    return nc


def round_fp32r(a):
    """Convert to fp16 for the PE fast path (full 2.4 GHz streaming rate)."""
    return np.ascontiguousarray(a, dtype=np.float16)


def make_core_inputs(x, w_attn, b_attn, w_proj):
    """Per-core input dicts; core index = batch*2 + head_group."""
    scale = np.float32(1.0 / np.sqrt(HD))
    tri = (
        np.arange(128)[:, None] <= np.arange(128)[None, :]
    ).astype(np.float16)
    sel = np.zeros((64, 128), dtype=np.float16)
    sel[0, 0:64] = 1.0
    sel[32, 64:128] = 1.0
    in_maps = []
    for b in range(NB):
        xTb = round_fp32r(x[b].T)
        for g in range(2):
            cols = slice(g * GC, (g + 1) * GC)
            w_qk = np.concatenate(
                [w_attn[:, cols] * scale, w_attn[:, C:][:, cols]], axis=1
            ).astype(np.float32)
            in_maps.append(
                {
                    "xT": xTb,
                    "w_qk": round_fp32r(w_qk),
                    "w_v": round_fp32r(w_attn[:, 2 * C :][:, cols]),
                    "b_q": np.ascontiguousarray(
                        b_attn[cols] * scale, dtype=np.float32
                    ),
                    "w_pr": round_fp32r(w_proj[cols, :]),
                    "tri16": tri,
                    "sel2": sel,
                }
            )
    return in_maps


_NC_CACHE = None


def get_nc():
    global _NC_CACHE
    if _NC_CACHE is None:
        nc = bacc.Bacc("TRN2", target_bir_lowering=False, debug=False)
        build_kernel(nc)
        nc.compile()
        _NC_CACHE = nc
    return _NC_CACHE


def kernel(x, w_attn, b_attn, w_proj, b_proj, _want_trace=False):
    x = np.asarray(x, dtype=np.float32)
    w_attn = np.asarray(w_attn, dtype=np.float32)
    b_attn = np.asarray(b_attn, dtype=np.float32)
    w_proj = np.asarray(w_proj, dtype=np.float32)
    b_proj = np.asarray(b_proj, dtype=np.float32)

    nc = get_nc()
    in_maps = make_core_inputs(x, w_attn, b_attn, w_proj)
    res = run_bass_kernel_spmd(
        nc, in_maps, list(range(8)), trace=_want_trace
    )
    bias_total = (b_attn[2 * C :] @ w_proj + b_proj).astype(np.float32)
    out = np.empty((NB, T, C), np.float32)
    for b in range(NB):
        out[b] = (
            res.results[2 * b]["yT"].T.astype(np.float32)
            + res.results[2 * b + 1]["yT"].T.astype(np.float32)
            + bias_total[None, :]
        )
    if _want_trace:
        return out, res
    return out
